# revision 1
# baseline (speedup 1.0000x reference)
"""Trainium2 Bass kernel for the text-CNN problem (dense_cnn).

Model: h = emb[x].reshape(B,1,L); three 1-channel 1D convs (K=3,4,5, 100
filters each) + bias + ReLU + global max-pool; concat; FC -> [B, 10].

Key identity: max_i relu(conv_i + b) == relu(b + max_i conv_i), so the
device only needs the raw per-filter max of each conv over all positions.

Device mapping (per core, 8-way shard over the 900k position axis):
  - conv as matmul: stationary [36, 128] packs 4 filters x 32 positions
    (Toeplitz bands, m = f_local*32 + r, entry [r+k, m] = w[f, 0, k]);
    moving operand is a stride-32 im2col of the signal: rhs[t, n] =
    sig[32*n + t], t in [0,36). One matmul column -> 128 useful outputs.
  - per (group, batch) "pack": 4 PSUM tiles [128, 896/862] (2-bank
    slots, 4-deep rotation over all 8 banks), 2 matmuls each.
  - drain: ScalarE copies tiles T0/T2 to SBUF bf16; DVE runs two
    independent tensor_tensor_scan(max, max) ops, each consuming one PSUM
    element and one SBUF element per cycle; each scan broadcast-writes its
    state onto one acc cell (last write wins = pair max) -> one DMA of
    acc[128, 300]; host maxes the column pairs.
Host: embedding gather, im2col prep (bf16), stationaries, final max over
r/cores, ragged-tail positions, ReLU+bias, FC.
"""

import os
import numpy as np

import concourse.bass as bass
import concourse.bacc as bacc
import concourse.mybir as mybir
from concourse.tile import TileContext
from concourse import bass_utils

import ml_dtypes

BF16 = ml_dtypes.bfloat16

# ---- problem constants (hardcoded; kernel.py must be self-contained) ----
VOCAB = 35097
WORD_DIM = 300
MAX_SENT = 3000
L = WORD_DIM * MAX_SENT          # 900000
B = 2
N_FILT = 100
KS = (3, 4, 5)
N_CLASSES = 10

N_CORES = 8
S = 32                            # positions per matmul column
TROWS = 36                        # S + max(K) - 1
GF = 4                            # filters per group
N_GROUPS = 3 * N_FILT // GF       # 75
TWS = (896, 896, 862, 862)        # PSUM tile widths (2-bank slots); the
                                  # two scan pairs are size-matched
NCOL_B = sum(TWS)                 # 3516 columns per batch (= ceil(112500/32))
NCOL = 2 * NCOL_B                 # 7032 columns per core
P5 = L - 5 + 1                    # 899996 valid positions for K=5
CHUNK = 112500                    # positions per core (8*112500 >= P5)
CSTART_MAX = P5 - S               # 899964 max column start

ACC_COLS = N_GROUPS * 4           # 300: two accum cols per (group, batch)


def _build_bass(n_groups=N_GROUPS, in_dt=mybir.dt.bfloat16):
    """Build the SPMD Bass module (same program on all cores).

    Per (group, batch): 4 PSUM tiles widths TWS (T0..T3; 2-bank slots, 8
    banks total, 4-slot rotation). ScalarE copies T0->cb0, T2->cb2 (bf16);
    DVE runs two independent tensor_tensor_scan(max, max) ops -- each
    consumes one PSUM and one SBUF element per cycle; each scan broadcast-
    writes its state onto one acc cell (last write = that pair's max).
    """
    nc = bacc.Bacc("TRN2", target_bir_lowering=False, debug=False,
                   num_devices=N_CORES)
    ncol = NCOL
    rhs_d = nc.dram_tensor("rhs", [TROWS, ncol], in_dt, kind="ExternalInput")
    wts_d = nc.dram_tensor("wts", [TROWS, n_groups * 128], in_dt,
                           kind="ExternalInput")
    acc_d = nc.dram_tensor("acc", [128, n_groups * 4], mybir.dt.float32,
                           kind="ExternalOutput")

    bf16 = mybir.dt.bfloat16
    MAX = mybir.AluOpType.max

    with TileContext(nc) as tc:
        with tc.tile_pool(name="io", bufs=1) as io_pool, \
             tc.tile_pool(name="cb", bufs=4) as c_pool, \
             tc.tile_pool(name="ps", bufs=4, space="PSUM") as psum_pool:
            rhs = io_pool.tile([TROWS, ncol], in_dt)
            wts = io_pool.tile([TROWS, n_groups * 128], in_dt)
            acc = io_pool.tile([128, n_groups * 4], mybir.dt.float32)
            nc.sync.dma_start(rhs[:, :], rhs_d[:, :])
            nc.sync.dma_start(wts[:, :], wts_d[:, :])
            tc.strict_bb_all_engine_barrier()

            for g in range(n_groups):
                lhsT = wts[:, g * 128:(g + 1) * 128]
                for b in range(2):
                    col0 = b * NCOL_B           # rhs col base for this batch
                    c0 = g * 2 + b
                    tiles = []
                    toff = 0
                    for t, tw in enumerate(TWS):
                        ps = psum_pool.tile([128, tw], mybir.dt.float32,
                                            tag="ps")
                        for jo, jn in ((0, 512), (512, tw - 512)):
                            o = col0 + toff + jo
                            nc.tensor.matmul(
                                ps[:, jo:jo + jn], lhsT,
                                rhs[:, o:o + jn], start=True, stop=True)
                        tiles.append(ps)
                        toff += tw

                    for pair in range(2):
                        tw = TWS[2 * pair]
                        cb = c_pool.tile([128, tw], bf16, tag="cbuf")
                        nc.scalar.copy(cb[:, :], tiles[2 * pair][:, :])
                        # scan state broadcast-writes one cell; the last
                        # write is the running max of both streams
                        dst = acc[:, 2 * c0 + pair:2 * c0 + pair + 1]
                        init = -3.0e38
                        nc.vector.tensor_tensor_scan(
                            dst.broadcast_to([128, tw]),
                            tiles[2 * pair + 1][:, :], cb[:, :],
                            init, op0=MAX, op1=MAX)

            nc.sync.dma_start(acc_d[:, :], acc[:, :])
    nc.compile()
    return nc


# ---------------- host-side preparation ----------------

def _build_stationary(w1, w2, w3):
    """[TROWS, N_GROUPS*128]: group g covers filters 4g..4g+3 of its conv,
    column m = f_local*32 + r, entry [r+k, m] = w[f, 0, k]."""
    ws = np.zeros((TROWS, N_GROUPS * 128), np.float32)
    convs = [(np.asarray(w1, np.float32), 3),
             (np.asarray(w2, np.float32), 4),
             (np.asarray(w3, np.float32), 5)]
    g = 0
    for w, K in convs:
        for g_local in range(N_FILT // GF):
            for fl in range(GF):
                f = g_local * GF + fl
                for r in range(S):
                    ws[r:r + K, g * 128 + fl * S + r] = w[f, 0, :]
            g += 1
    return ws


def _column_starts(core):
    base = core * CHUNK
    starts = base + S * np.arange(NCOL_B)
    return np.minimum(starts, CSTART_MAX)


def _make_rhs(sig, core, dtype):
    """sig: [B, L] fp32 -> [TROWS, 2*NCOL_B] im2col for this core."""
    starts = _column_starts(core)
    cols = []
    for b in range(B):
        win = np.lib.stride_tricks.sliding_window_view(sig[b], TROWS)
        cols.append(win[starts].T)          # [TROWS, NCOL_B]
    return np.ascontiguousarray(np.concatenate(cols, axis=1)).astype(dtype)


_CACHE = {}


def _get_nc():
    if "nc" not in _CACHE:
        _CACHE["nc"] = _build_bass()
    return _CACHE["nc"]


def _device_acc(rhs_list, wts):
    """Run the bass kernel on 8 cores. rhs_list[i]: [TROWS, 2*NCOL_B].
    Returns list of acc arrays [128, ACC_COLS] fp32."""
    if os.environ.get("KERNEL_EMULATE"):
        out = []
        for rhs in rhs_list:
            acc = np.empty((128, ACC_COLS), np.float32)
            for g in range(N_GROUPS):
                pg = np.einsum("tm,tn->mn",
                               wts[:, g * 128:(g + 1) * 128].astype(np.float32),
                               rhs.astype(np.float32))  # [128, 2*NCOL_B]
                half = TWS[0] + TWS[1]
                for b in range(2):
                    seg = pg[:, b * NCOL_B:(b + 1) * NCOL_B]
                    acc[:, 4 * g + 2 * b] = seg[:, :half].max(axis=1)
                    acc[:, 4 * g + 2 * b + 1] = seg[:, half:].max(axis=1)
            out.append(acc)
        return out

    nc = _get_nc()
    in_maps = [{"rhs": rhs, "wts": wts} for rhs in rhs_list]
    res = bass_utils.run_bass_kernel_spmd(nc, in_maps,
                                          core_ids=list(range(N_CORES)))
    return [r["acc"] for r in res.results]


def kernel(x, emb, w1, b1, w2, b2, w3, b3, fc_w, fc_b):
    x = np.asarray(x)
    emb = np.asarray(emb, np.float32)
    sig = emb[x.reshape(-1)].reshape(B, L)          # [2, 900000] fp32

    wts = _build_stationary(w1, w2, w3).astype(BF16)
    rhs_list = [_make_rhs(sig, c, BF16) for c in range(N_CORES)]

    accs = _device_acc(rhs_list, wts)

    # acc[m, g*NBLK + blk]; blocks 0..6 batch0, 7..13 batch1
    # -> per-batch per-filter maxes
    conv_max = np.full((B, 3 * N_FILT), -np.inf, np.float32)
    for acc in accs:
        a = acc.reshape(128, N_GROUPS, 2, 2)
        for b in range(B):
            mb = a[:, :, b, :].max(axis=2)                  # [128, 75]
            # rows m = f_local*32 + r -> [GF, S, N_GROUPS] -> max over r
            mb = mb.reshape(GF, S, N_GROUPS).max(axis=1)           # [GF, 75]
            # filter id = group_base + (g_local*GF + f_local)
            mb = mb.T.reshape(3, N_FILT // GF, GF).reshape(3 * N_FILT)
            conv_max[b] = np.maximum(conv_max[b], mb)

    # ragged tail positions not covered on device (fp32 host math)
    w1a = np.asarray(w1, np.float32)
    w2a = np.asarray(w2, np.float32)
    for b in range(B):
        for p in (L - 3 + 1 - 1, L - 3 + 1 - 2):   # 899997, 899996 (K=3)
            if p > P5 - 1:
                v = sig[b, p:p + 3] @ w1a[:, 0, :].T
                conv_max[b, :N_FILT] = np.maximum(conv_max[b, :N_FILT], v)
        p = L - 4 + 1 - 1                           # 899996 (K=4)
        if p > P5 - 1:
            v = sig[b, p:p + 4] @ w2a[:, 0, :].T
            conv_max[b, N_FILT:2 * N_FILT] = \
                np.maximum(conv_max[b, N_FILT:2 * N_FILT], v)

    bias = np.concatenate([np.asarray(b1, np.float32),
                           np.asarray(b2, np.float32),
                           np.asarray(b3, np.float32)])
    feats = np.maximum(conv_max + bias[None, :], 0.0)
    out = feats @ np.asarray(fc_w, np.float32).T + np.asarray(fc_b, np.float32)
    return out.astype(np.float32)



# revision 17
# speedup vs baseline: 1.0012x; 1.0012x over previous
"""Trainium2 Bass kernel for the text-CNN problem (dense_cnn).

Model: h = emb[x].reshape(B,1,L); three 1-channel 1D convs (K=3,4,5, 100
filters each) + bias + ReLU + global max-pool; concat; FC -> [B, 10].

Key identity: max_i relu(conv_i + b) == relu(b + max_i conv_i), so the
device only needs the raw per-filter max of each conv over all positions.

Device mapping (per core, 8-way shard over the 900k position axis):
  - conv as matmul: fp8 DoubleRow perf mode (0.5 cycles per output
    column).  Stationary [72, 2, 128] packs 4 filters x 32 positions
    (Toeplitz bands, m = f_local*32 + r).  Contraction is error-corrected
    3-term fp8: half0 = [s8; s8] x [w8; dw8], half1 = [r8; 0] x [w8/8; 0]
    where s8 = e4m3(32 s), r8 = e4m3(8 (32s - s8)), w8 = e4m3(32 w),
    dw8 = e4m3(8 (32w - w8))/8.  Matmul result = 1024*conv to ~0.3%.
  - drain (the bottleneck; PSUM exits limited to DVE+Act):
    per (group, batch) two rounds of 1758 columns in one rotating PSUM
    tile [128, 1758] (bufs=2).  Act copies region [0:967) to SBUF bf16;
    DVE runs tensor_tensor_scan(max,max) pairing PSUM region [967:1758)
    with the first 791 copied columns; Pool (gpsimd cannot touch PSUM)
    pair-scans the remaining 176 copied columns.  Each scan broadcast-
    writes its running state onto one acc cell (last write wins).
Host: embedding gather, fp8 quantization + im2col, stationaries, final
max over cells/r/cores, ragged-tail positions, ReLU+bias, FC.
"""

import os
import numpy as np

import concourse.bass as bass
import concourse.bacc as bacc
import concourse.mybir as mybir
from concourse.tile import TileContext
from concourse import bass_utils

import ml_dtypes

BF16 = ml_dtypes.bfloat16
E4M3 = ml_dtypes.float8_e4m3

# ---- problem constants (hardcoded; kernel.py must be self-contained) ----
VOCAB = 35097
WORD_DIM = 300
MAX_SENT = 3000
L = WORD_DIM * MAX_SENT          # 900000
B = 2
N_FILT = 100
KS = (3, 4, 5)
N_CLASSES = 10

N_CORES = 8
S = 32                            # positions per matmul column
TROWS = 36                        # S + max(K) - 1
PPART = 72                        # contraction partitions (2 band sets)
GF = 4                            # filters per group
N_GROUPS = 3 * N_FILT // GF       # 75
NCOL_B = 3516                     # ceil(112500/32) columns per batch
NCOL = 2 * NCOL_B                 # 7032 columns per core
P5 = L - 5 + 1                    # 899996 valid positions for K=5
CHUNK = 112500                    # positions per core
CSTART_MAX = P5 - S               # max column start

# drain geometry: per (g,b) four tiles [A1 | D1 | A2 | D2] in a 4-slot
# rotation (2 PSUM banks per slot).  Act copies A-tiles to SBUF bf16;
# DVE pair-scans (D-tile, cb).  (The Pool/gpsimd engine cannot execute
# compute instructions in this toolchain, and PSUM exits are limited to
# DVE+Act, so the even split is the bandwidth optimum.)
AW = 879                          # Act-copied tile width
DW = 879                          # DVE pair width
N_ROUNDS = 2                      # (A, D) units per (g, b)
CELLS = N_ROUNDS                  # acc cells per (g,b): dve x2
ACC_COLS = N_GROUPS * B * CELLS   # 300

SS = 32.0                         # signal scale
SW = 32.0                         # weight scale
RS = 8.0                          # residual scale

MMC = 256                         # out columns per DoubleRow matmul


def _build_bass():
    """SPMD Bass module (same program on all cores)."""
    nc = bacc.Bacc("TRN2", target_bir_lowering=False, debug=False,
                   num_devices=N_CORES)
    fp8 = mybir.dt.float8e4
    f32 = mybir.dt.float32
    bf16 = mybir.dt.bfloat16
    MAX = mybir.AluOpType.max
    DR = mybir.MatmulPerfMode.DoubleRow

    rhs_d = nc.dram_tensor("rhs", [PPART, 2, NCOL], fp8, kind="ExternalInput")
    wts_d = nc.dram_tensor("wts", [PPART, 2, N_GROUPS * 128], fp8,
                           kind="ExternalInput")
    acc_d = nc.dram_tensor("acc", [128, ACC_COLS], f32, kind="ExternalOutput")

    with TileContext(nc) as tc:
        with tc.tile_pool(name="io", bufs=1) as io_pool, \
             tc.tile_pool(name="cb", bufs=3) as c_pool, \
             tc.tile_pool(name="ps", bufs=4, space="PSUM") as psum_pool:
            rhs = io_pool.tile([PPART, 2, NCOL], fp8)
            wts = io_pool.tile([PPART, 2, N_GROUPS * 128], fp8)
            acc = io_pool.tile([128, ACC_COLS], f32)
            # interleave input DMAs so group-0/batch-0 data lands first;
            # spread the first three across queues to overlap desc-gen
            WCH = 19 * 128
            nc.sync.dma_start(wts[:, :, 0:WCH], wts_d[:, :, 0:WCH])
            nc.scalar.dma_start(rhs[:, 0, 0:NCOL_B], rhs_d[:, 0, 0:NCOL_B])
            nc.sync.dma_start(rhs[:, 1, 0:NCOL_B], rhs_d[:, 1, 0:NCOL_B])
            for wo in range(WCH, N_GROUPS * 128, WCH):
                wn = min(WCH, N_GROUPS * 128 - wo)
                nc.sync.dma_start(wts[:, :, wo:wo + wn],
                                  wts_d[:, :, wo:wo + wn])
            nc.sync.dma_start(rhs[:, 0, NCOL_B:NCOL], rhs_d[:, 0, NCOL_B:NCOL])
            nc.sync.dma_start(rhs[:, 1, NCOL_B:NCOL], rhs_d[:, 1, NCOL_B:NCOL])

            for g in range(N_GROUPS):
                lhsT = wts[:, :, g * 128:(g + 1) * 128]
                for b in range(B):
                    col0 = b * NCOL_B
                    for r in range(N_ROUNDS):
                        base = col0 + r * (AW + DW)
                        gbase = (g * B + b) * CELLS
                        pa = psum_pool.tile([128, AW], f32, tag="ps")
                        for o in range(0, AW, MMC):
                            n = min(MMC, AW - o)
                            nc.tensor.matmul(
                                pa[:, o:o + n], lhsT,
                                rhs[:, :, base + o:base + o + n],
                                start=True, stop=True, perf_mode=DR)
                        cb = c_pool.tile([128, AW], bf16, tag="cbuf")
                        nc.scalar.copy(cb[:, :], pa[:, :])
                        pd = psum_pool.tile([128, DW], f32, tag="ps")
                        for o in range(0, DW, MMC):
                            n = min(MMC, DW - o)
                            nc.tensor.matmul(
                                pd[:, o:o + n], lhsT,
                                rhs[:, :, base + AW + o:base + AW + o + n],
                                start=True, stop=True, perf_mode=DR)
                        # DVE: pair (PSUM D-tile, cb) -> cell
                        nc.vector.tensor_tensor_scan(
                            acc[:, gbase + r:gbase + r + 1]
                               .broadcast_to([128, DW]),
                            pd[:, :], cb[:, 0:DW],
                            -3.0e38, op0=MAX, op1=MAX)

            tc.strict_bb_all_engine_barrier()
            nc.sync.dma_start(acc_d[:, :], acc[:, :])
    nc.compile()
    return nc


# ---------------- host-side preparation ----------------

def _q8(x):
    return np.clip(np.asarray(x, np.float32), -240.0, 240.0).astype(E4M3)


def _quant_weights(w1, w2, w3):
    """Returns (w8, dw, w8_8): each [300, 5] fp32 (padded bands)."""
    w = np.zeros((3 * N_FILT, 5), np.float32)
    for i, (wa, K) in enumerate(((w1, 3), (w2, 4), (w3, 5))):
        w[i * N_FILT:(i + 1) * N_FILT, :K] = np.asarray(wa, np.float32)[:, 0, :]
    w8 = _q8(SW * w).astype(np.float32)
    dw = _q8(RS * (SW * w - w8)).astype(np.float32) / RS
    w8_8 = w8 / RS
    return w8, _q8(dw).astype(np.float32), _q8(w8_8).astype(np.float32)


def _build_stationary(w1, w2, w3):
    """[PPART, 2, N_GROUPS*128] fp8: group g covers filters 4g..4g+3,
    column m = f_local*32 + r; half0 rows 0:36 w8 bands, rows 36:72 dw
    bands; half1 rows 0:36 w8/8 bands."""
    w8, dw, w8_8 = _quant_weights(w1, w2, w3)
    ws = np.zeros((PPART, 2, N_GROUPS * 128), np.float32)
    for g in range(N_GROUPS):
        conv_i = (g * GF) // N_FILT
        K = KS[conv_i]
        for fl in range(GF):
            f = g * GF + fl
            band8 = w8[f, :K]
            bandd = dw[f, :K]
            band8_8 = w8_8[f, :K]
            for r in range(S):
                m = g * 128 + fl * S + r
                ws[r:r + K, 0, m] = band8
                ws[36 + r:36 + r + K, 0, m] = bandd
                ws[r:r + K, 1, m] = band8_8
    return ws.astype(E4M3)


def _column_starts(core):
    base = core * CHUNK
    starts = base + S * np.arange(NCOL_B)
    return np.minimum(starts, CSTART_MAX)


def _quant_signal(sig):
    """sig [B, L] fp32 -> (s8, r8) fp32 arrays."""
    s8 = _q8(SS * sig).astype(np.float32)
    r8 = _q8(RS * (SS * sig - s8)).astype(np.float32)
    return s8, r8


def _make_rhs(s8, r8, core):
    """[PPART, 2*NCOL] fp8 im2col for this core.
    Layout per partition row: [half0 (NCOL) | half1 (NCOL)].
    rows 0:36 half0 = s8 windows, half1 = r8 windows;
    rows 36:72 half0 = s8 dup, half1 = 0."""
    starts = _column_starts(core)
    out = np.zeros((PPART, 2, NCOL), np.float32)
    for b in range(B):
        win_s = np.lib.stride_tricks.sliding_window_view(s8[b], TROWS)
        win_r = np.lib.stride_tricks.sliding_window_view(r8[b], TROWS)
        cs = win_s[starts].T                      # [36, NCOL_B]
        cr = win_r[starts].T
        sl = slice(b * NCOL_B, (b + 1) * NCOL_B)
        out[0:36, 0, sl] = cs
        out[36:72, 0, sl] = cs
        out[0:36, 1, sl] = cr
    return np.ascontiguousarray(out).astype(E4M3)


_CACHE = {}


def _get_nc():
    if "nc" not in _CACHE:
        _CACHE["nc"] = _build_bass()
    return _CACHE["nc"]


def _device_acc(rhs_list, wts):
    """Run the bass kernel on 8 cores; returns [128, ACC_COLS] fp32 list."""
    if os.environ.get("KERNEL_EMULATE"):
        out = []
        wv = wts.astype(np.float32)
        for rhs in rhs_list:
            rv = rhs.astype(np.float32)
            acc = np.full((128, ACC_COLS), -3.0e38, np.float32)
            for g in range(N_GROUPS):
                wg = wv[:, :, g * 128:(g + 1) * 128]
                pg = (wg[:, 0].T @ rv[:, 0] + wg[:, 1].T @ rv[:, 1])
                for b in range(B):
                    gbase = (g * B + b) * CELLS
                    for r in range(N_ROUNDS):
                        base = b * NCOL_B + r * (AW + DW)
                        seg = pg[:, base:base + AW + DW]
                        cbv = seg[:, 0:AW].astype(BF16).astype(np.float32)
                        acc[:, gbase + r] = np.maximum(
                            seg[:, AW:AW + DW].max(axis=1),
                            cbv[:, 0:DW].max(axis=1))
            out.append(acc)
        return out

    nc = _get_nc()
    in_maps = [{"rhs": rhs, "wts": wts} for rhs in rhs_list]
    res = bass_utils.run_bass_kernel_spmd(nc, in_maps,
                                          core_ids=list(range(N_CORES)))
    return [r["acc"] for r in res.results]


def kernel(x, emb, w1, b1, w2, b2, w3, b3, fc_w, fc_b):
    x = np.asarray(x)
    emb = np.asarray(emb, np.float32)
    sig = emb[x.reshape(-1)].reshape(B, L)          # [2, 900000] fp32

    wts = _build_stationary(w1, w2, w3)
    s8, r8 = _quant_signal(sig)
    rhs_list = [_make_rhs(s8, r8, c) for c in range(N_CORES)]

    accs = _device_acc(rhs_list, wts)

    # acc[m, (g*B + b)*CELLS + cell] ; rows m = f_local*32 + r
    conv_max = np.full((B, 3 * N_FILT), -np.inf, np.float32)
    for acc in accs:
        a = acc.reshape(128, N_GROUPS, B, CELLS).max(axis=3)   # [128, 75, 2]
        for b in range(B):
            mb = a[:, :, b]                                    # [128, 75]
            mb = mb.reshape(GF, S, N_GROUPS).max(axis=1)       # [GF, 75]
            mb = mb.T.reshape(N_GROUPS * GF)                   # filter id
            conv_max[b] = np.maximum(conv_max[b], mb)
    conv_max /= (SS * SW)

    # ragged tail positions not covered on device (fp32 host math)
    w1a = np.asarray(w1, np.float32)
    w2a = np.asarray(w2, np.float32)
    for b in range(B):
        for p in (L - 3 + 1 - 1, L - 3 + 1 - 2):   # (K=3)
            if p > P5 - 1:
                v = sig[b, p:p + 3] @ w1a[:, 0, :].T
                conv_max[b, :N_FILT] = np.maximum(conv_max[b, :N_FILT], v)
        p = L - 4 + 1 - 1                           # (K=4)
        if p > P5 - 1:
            v = sig[b, p:p + 4] @ w2a[:, 0, :].T
            conv_max[b, N_FILT:2 * N_FILT] = \
                np.maximum(conv_max[b, N_FILT:2 * N_FILT], v)

    bias = np.concatenate([np.asarray(b1, np.float32),
                           np.asarray(b2, np.float32),
                           np.asarray(b3, np.float32)])
    feats = np.maximum(conv_max + bias[None, :], 0.0)
    out = feats @ np.asarray(fc_w, np.float32).T + np.asarray(fc_b, np.float32)
    return out.astype(np.float32)
